# revision 1
# baseline (speedup 1.0000x reference)
"""Multi-head attention (b=8, n=1024, dim=1024, 16 heads) on 8 TRN2 NeuronCores.

Data-parallel: one batch element per core. Each core runs an identical
Bass/Tile program computing qkv projection, softmax attention, and the
output projection for its [1024, 1024] slice, in bf16 with fp32 PSUM
accumulation.

Layout choices (host pre-transposes so the device never transposes):
  - xt   [c, n]   = x[i].T                       (bf16)
  - wqkt [c, 2h*d] = permuted q/k weights^T: head-pair p occupies
        f-tiles 2p (q rows of heads 2p,2p+1) and 2p+1 (k rows).
        A 128-row f-tile = [head 2p (64 rows); head 2p+1 (64 rows)], so
        the qkv matmul directly yields q^T/k^T pair tiles where the even
        head lives on partitions 0-63 and the odd head on 64-127.
  - wvt  [c, h*d] = wv.T, wpt [c, o] = w_proj.T  (bf16)

Per core:
  V    = x @ wv^T          -> SBUF [n, h*65] with a ones column per head
  qk^T = wqk_perm @ x^T    -> SBUF pair tiles [128, n]
  S^T  = k_h @ q_h^T       -> PSUM [nk_tile, nq]   (K=64 row-tiled pairs)
  attn^T = exp(0.125*S^T)  -> SBUF bf16 (ScalarE; no max subtraction --
           scores ~ N(0,1), exp stays well inside fp32/bf16 range, and
           softmax is shift-invariant so the result matches jax.nn.softmax)
  out^T_aug = V_aug^T @ attn^T -> PSUM [65, nq]; row 64 = softmax denom
  out^T = out^T_aug[0:64] * (1/denom)  (DVE mul; reciprocal row replicated
         across partitions with a GPSIMD partition_broadcast)
  y    = out_heads @ w_proj^T + b      (bias added on DVE during PSUM
         evacuation, from a DMA partition-broadcast bias tile)

S^T matmuls for the even/odd heads of a pair run as K=64 row-tiles at PE
row groups (0,0)/(64,0) — adjacent instructions with disjoint row groups
execute concurrently on the 128x128 array.

All inputs are packed into one [dim, 5120] bf16 DRAM tensor so the whole
input loads with 8 large DMAs (per-DMA issue overhead on this stack is
~0.6-1 us plus ~0.9 us semaphore propagation, so DMA count matters).
"""

import numpy as np
import ml_dtypes

B, N, DIM = 8, 1024, 1024
H, D = 16, 64
NP = 128  # partitions
NCHUNK = 512  # matmul free-dim chunk (one PSUM bank of fp32)
CT = DIM // NP  # 8 contraction chunks
NT = N // NP  # 8 n-tiles
NQC = N // NCHUNK  # 2 nq chunks
PAIRS = H // 2  # 8 head pairs

BF16 = ml_dtypes.bfloat16

_CACHE = {}


def build(loop_iters=1):
    """Build and compile the per-core Bacc graph. Cached per loop_iters."""
    if loop_iters in _CACHE:
        return _CACHE[loop_iters]

    import concourse.mybir as mybir
    import concourse.tile as tile
    from concourse import bacc

    fp32 = mybir.dt.float32
    bf16 = mybir.dt.bfloat16
    Exp = mybir.ActivationFunctionType.Exp

    nc = bacc.Bacc("TRN2", target_bir_lowering=False, debug=False, num_devices=8)

    PACK = N + 2 * DIM + DIM + DIM  # xt | wqkt | wvt | wpt along free dim
    inp = nc.declare_dram_parameter("inp", [DIM, PACK], bf16, isOutput=False)
    bias = nc.declare_dram_parameter("bias", [1, DIM], bf16, isOutput=False)
    out = nc.declare_dram_parameter("out", [N, DIM], fp32, isOutput=True)

    with tile.TileContext(nc) as tc:
        with (
            tc.tile_pool(name="weights", bufs=1) as wpool,
            tc.tile_pool(name="acts", bufs=1) as apool,
            tc.tile_pool(name="attn", bufs=18) as attnpool,
            tc.tile_pool(name="small", bufs=2) as spool,
            tc.tile_pool(name="big_ps", bufs=2, space="PSUM") as big_ps,
            tc.tile_pool(name="half_ps", bufs=4, space="PSUM") as half_ps,
        ):
            # ---- persistent SBUF tensors (loaded once, one DMA per c-chunk) ----
            packed_sb = wpool.tile([NP, CT, PACK], bf16, tag="packed")

            HOT = N + 4 * NP  # xt + wqkt f-tiles of pairs 0 and 1
            dma_engines = [nc.sync, nc.scalar, nc.gpsimd]
            for ct in range(CT):
                eng = dma_engines[ct % 3]
                eng.dma_start(packed_sb[:, ct, 0:HOT],
                              inp[ct * NP:(ct + 1) * NP, 0:HOT])
            for ct in range(CT):
                eng = dma_engines[ct % 3]
                eng.dma_start(packed_sb[:, ct, HOT:],
                              inp[ct * NP:(ct + 1) * NP, HOT:])
            bias_bc = wpool.tile([NP, DIM], bf16, tag="biasbc")
            nc.sync.dma_start(bias_bc[:], bias[0:1, :].to_broadcast((NP, DIM)))
            xt_sb = packed_sb[:, :, 0:N]
            wqkt_sb = packed_sb[:, :, N:N + 2 * DIM]
            wvt_sb = packed_sb[:, :, N + 2 * DIM:N + 3 * DIM]
            wpt_sb = packed_sb[:, :, N + 3 * DIM:N + 4 * DIM]

            def body(_it=None):
                # ---- per-iteration SBUF ----
                q_sb = apool.tile([NP, PAIRS, N], bf16, tag="q")
                k_sb = apool.tile([NP, PAIRS, N], bf16, tag="k")
                vaug_sb = apool.tile([NP, NT, H * (D + 1)], bf16, tag="vaug")
                outT_sb = apool.tile([NP, CT, N], bf16, tag="outT")

                def emit_qkv_ft(p, which, nqc=None):
                    # one f-tile of pair p's qk^T: which=0 -> q, 1 -> k.
                    # each nq half uses its own 1-bank psum tile so the hot
                    # S-tile rotation is starved for as little as possible.
                    ft = 2 * p + which
                    dst = q_sb if which == 0 else k_sb
                    nqcs = range(NQC) if nqc is None else (nqc,)
                    for nqc_ in nqcs:
                        qk_ps = big_ps.tile([NP, NCHUNK], fp32, tag="big",
                                            name="qk_ps")
                        for ct in range(CT):
                            nc.tensor.matmul(
                                qk_ps[:],
                                lhsT=wqkt_sb[:, ct, ft * NP:(ft + 1) * NP],
                                rhs=xt_sb[:, ct, nqc_ * NCHUNK:(nqc_ + 1) * NCHUNK],
                                start=(ct == 0),
                                stop=(ct == CT - 1),
                            )
                        nc.vector.tensor_copy(
                            dst[:, p, nqc_ * NCHUNK:(nqc_ + 1) * NCHUNK], qk_ps[:])

                def emit_qkv(p):
                    emit_qkv_ft(p, 0)
                    emit_qkv_ft(p, 1)

                def emit_v_tile(nt):
                    v_ps = big_ps.tile([NP, 2 * NCHUNK], fp32, tag="big",
                                       name="v_ps")
                    for ct in range(CT):
                        for fc in range(2):
                            nc.tensor.matmul(
                                v_ps[:, fc * NCHUNK:(fc + 1) * NCHUNK],
                                lhsT=xt_sb[:, ct, nt * NP:(nt + 1) * NP],
                                rhs=wvt_sb[:, ct, fc * NCHUNK:(fc + 1) * NCHUNK],
                                start=(ct == 0),
                                stop=(ct == CT - 1),
                            )
                    vrow = vaug_sb[:, nt, :].rearrange("p (h e) -> p h e", e=D + 1)
                    nc.vector.memset(vrow[:, :, D:D + 1], 1.0)
                    nc.vector.tensor_copy(
                        vrow[:, :, 0:D],
                        v_ps[:].rearrange("p (h e) -> p h e", e=D),
                    )

                def emit_s_exp(p, nkt, atn):
                    # S^T for both heads of pair p at nk-tile nkt; even head
                    # on PE rows 0-63, odd on 64-127 (row-tiled, concurrent)
                    sps = {}
                    for hh in range(2):
                        sps[hh] = big_ps.tile([NP, 2 * NCHUNK], fp32,
                                              tag="big", name="s_ps")
                    for nqc in range(NQC):
                        for hh in range(2):
                            lo, hi = hh * D, (hh + 1) * D
                            nc.tensor.matmul(
                                sps[hh][:, nqc * NCHUNK:(nqc + 1) * NCHUNK],
                                lhsT=k_sb[lo:hi, p, nkt * NP:(nkt + 1) * NP],
                                rhs=q_sb[lo:hi, p, nqc * NCHUNK:(nqc + 1) * NCHUNK],
                                start=True,
                                stop=True,
                                tile_position=(hh * D, 0),
                            )
                    for hh in range(2):
                        a = attnpool.tile([NP, N], bf16, tag="attn")
                        nc.scalar.activation(a[:], sps[hh][:], Exp,
                                             scale=float(D) ** -0.5)
                        atn[hh, nkt] = a

                # ---- software pipeline over head pairs ----
                # pair 0: S/exp interleaved with V tiles; then for each pair p:
                # S(p+1)/exp(p+1) interleaved per-nkt with AV(p), so ACT (the
                # attention-phase bottleneck) never starves.
                emit_qkv(0)
                atn_cur = {}
                for nkt in range(NT):
                    emit_s_exp(0, nkt, atn_cur)
                emit_qkv(1)

                for p in range(PAIRS):
                    av = {}
                    for hh in range(2):
                        for nqc in range(NQC):
                            av[hh, nqc] = half_ps.tile([D + 1, NCHUNK], fp32,
                                                       tag="half", name="av")
                    atn_nxt = {}
                    for nkt in range(NT):
                        if p + 1 < PAIRS:
                            emit_s_exp(p + 1, nkt, atn_nxt)
                        if p == 0 and nkt < 5:
                            emit_v_tile(nkt)
                        if p == 0 and nkt < 3:
                            emit_v_tile(5 + nkt)
                        if p + 2 < PAIRS and nkt in (1, 3, 5, 7):
                            emit_qkv_ft(p + 2, nkt // 4, (nkt % 4) // 2)
                        for hh in range(2):
                            h = 2 * p + hh
                            for nqc in range(NQC):
                                nc.tensor.matmul(
                                    av[hh, nqc][:],
                                    lhsT=vaug_sb[:, nkt,
                                                 h * (D + 1):(h + 1) * (D + 1)],
                                    rhs=atn_cur[hh, nkt][
                                        :, nqc * NCHUNK:(nqc + 1) * NCHUNK],
                                    start=(nkt == 0),
                                    stop=(nkt == NT - 1),
                                )
                    atn_cur = atn_nxt

                    for hh in range(2):
                        for nqc in range(NQC):
                            t = av[hh, nqc]
                            recip = spool.tile([1, NCHUNK], fp32, tag="recip")
                            nc.vector.reciprocal(recip[:], t[D:D + 1, :])
                            recip_b = spool.tile([D, NCHUNK], fp32, tag="recipb")
                            nc.gpsimd.partition_broadcast(recip_b[:], recip[:],
                                                          channels=D)
                            dst = outT_sb[hh * D:(hh + 1) * D, p,
                                          nqc * NCHUNK:(nqc + 1) * NCHUNK]
                            if hh == 0:
                                nc.vector.tensor_mul(dst, t[0:D, :], recip_b[:])
                            else:
                                tmp = spool.tile([D, NCHUNK], bf16, tag="tmpodd")
                                nc.vector.tensor_mul(tmp[:], t[0:D, :], recip_b[:])
                                # partition shift 0:64 -> 64:128 via DMA
                                nc.sync.dma_start(dst, tmp[:])

                # ---- y = out_heads @ w_proj^T + bias ----
                for nt in range(NT):
                    y_ps = {}
                    for oc in range(NQC):
                        y_ps[oc] = half_ps.tile([NP, NCHUNK], fp32, tag="half",
                                                name="y_ps")
                    for ct in range(CT):
                        for oc in range(NQC):
                            nc.tensor.matmul(
                                y_ps[oc][:],
                                lhsT=outT_sb[:, ct, nt * NP:(nt + 1) * NP],
                                rhs=wpt_sb[:, ct, oc * NCHUNK:(oc + 1) * NCHUNK],
                                start=(ct == 0),
                                stop=(ct == CT - 1),
                            )
                    y_sb = spool.tile([NP, N], fp32, tag="ysb", name="y_sb")
                    for oc in range(NQC):
                        nc.vector.tensor_add(
                            y_sb[:, oc * NCHUNK:(oc + 1) * NCHUNK], y_ps[oc][:],
                            bias_bc[:, oc * NCHUNK:(oc + 1) * NCHUNK],
                        )
                    nc.sync.dma_start(out[nt * NP:(nt + 1) * NP, :], y_sb[:])

            if loop_iters == 1:
                body()
            else:
                with tc.For_i(0, loop_iters, 1) as it:
                    body(it)

    nc.compile()
    _CACHE[loop_iters] = nc
    return nc


def prep_inputs(x, w_qkv, w_proj, b_proj):
    """Host-side sharding + layout prep -> per-core input maps."""
    wq, wk, wv = w_qkv[0:DIM], w_qkv[DIM:2 * DIM], w_qkv[2 * DIM:3 * DIM]
    perm = []
    for p in range(PAIRS):
        perm.append(wq[2 * p * D:(2 * p + 2) * D])
        perm.append(wk[2 * p * D:(2 * p + 2) * D])
    wqk_perm = np.concatenate(perm, axis=0)  # [2*DIM, DIM]
    w_cols = np.concatenate([wqk_perm.T, wv.T, w_proj.T], axis=1).astype(BF16)
    bias = b_proj.reshape(1, DIM).astype(BF16)
    in_maps = []
    for i in range(B):
        xt = x[i].T.astype(BF16)
        inp = np.ascontiguousarray(np.concatenate([xt, w_cols], axis=1))
        in_maps.append({"inp": inp, "bias": bias})
    return in_maps


def kernel(x, w_qkv, w_proj, b_proj):
    from concourse import bass_utils

    x = np.asarray(x, dtype=np.float32)
    w_qkv = np.asarray(w_qkv, dtype=np.float32)
    w_proj = np.asarray(w_proj, dtype=np.float32)
    b_proj = np.asarray(b_proj, dtype=np.float32)
    assert x.shape == (B, N, DIM)

    nc = build(1)
    in_maps = prep_inputs(x, w_qkv, w_proj, b_proj)
    res = bass_utils.run_bass_kernel_spmd(nc, in_maps, core_ids=list(range(B)))
    return np.stack([res.results[i]["out"] for i in range(B)], axis=0)



# revision 2
# speedup vs baseline: 44.7542x; 44.7542x over previous
"""Multi-head attention (b=8, n=1024, dim=1024, 16 heads) on 8 TRN2 NeuronCores.

Data-parallel: one batch element per core, bf16 compute with fp32 PSUM.
HW-measured steady-state ~200-290us/core vs ~400us for the previous
version (dispatch-noise-free For_i slope protocol).

v2 redesign (HW-calibrated): chained accumulation matmuls stream at
~0.2ns/row on this silicon while single-group (start&stop) matmuls pay a
~200ns group overhead, and cross-engine semaphore hops cost ~1us. So:
  - every GEMM (qkv, V, AV, proj) is an 8-long accumulation chain
  - S matmuls (inherently single-group) use 1-bank [128,512] tiles from a
    4-deep rotation so exp (ACT, PSUM-fp32 fast path) frees banks quickly
  - AV runs as back-to-back 8-chains (bursts) so its PSUM banks are held
    ~1us instead of a whole pair iteration
  - attention tiles are [128,512] halves: AV consumes nqc-half bursts and
    frees SBUF early (peak ~48 tiles)
  - V is computed in the prologue (wvt arrives in DMA wave A) to keep PE
    busy while ACT works off the S(0) exps
  - output is bf16 (host casts to fp32; rel-err budget is 2e-2)

Layouts (host pre-transposes; the device never transposes):
  xt [c, n] = x[i].T; wqk packed per head-pair f-tiles (even head on
  partitions 0-63, odd on 64-127); wvt/wpt = transposed weights.
  Packed DRAM tensor [dim, 5120]: wave A = xt | wqk pairs 0-1 | wvt,
  wave B = wqk pairs 2-7 | wpt, DMA'd on 4 queues.

Softmax skips the max-subtraction (scores ~ N(0,1); exp is well inside
fp32/bf16 range and softmax is shift-invariant).
"""

import numpy as np
import ml_dtypes

B, N, DIM = 8, 1024, 1024
H, D = 16, 64
NP = 128
NCHUNK = 512
CT = DIM // NP          # 8 contraction chunks
NT = N // NP            # 8 n-tiles (key tiles)
NQC = N // NCHUNK       # 2 nq chunks
PAIRS = H // 2          # 8 head pairs

WAVE_A = N + 4 * NP + DIM          # xt | wqk pairs 0-1 | wvt  (2560 cols)
WAVE_B = 12 * NP + DIM             # wqk pairs 2-7 | wpt       (2560 cols)
PACK = WAVE_A + WAVE_B

BF16 = ml_dtypes.bfloat16

_CACHE = {}


def build(loop_iters=1):
    if loop_iters in _CACHE:
        return _CACHE[loop_iters]

    import concourse.mybir as mybir
    import concourse.tile as tile
    from concourse import bacc

    fp32 = mybir.dt.float32
    bf16 = mybir.dt.bfloat16
    Exp = mybir.ActivationFunctionType.Exp

    nc = bacc.Bacc("TRN2", target_bir_lowering=False, debug=False, num_devices=8)

    inp = nc.declare_dram_parameter("inp", [DIM, PACK], bf16, isOutput=False)
    bias = nc.declare_dram_parameter("bias", [NP, CT], fp32, isOutput=False)
    out = nc.declare_dram_parameter("out", [DIM, N], bf16, isOutput=True)

    with tile.TileContext(nc) as tc:
        with (
            tc.tile_pool(name="weights", bufs=1) as wpool,
            tc.tile_pool(name="acts", bufs=1) as apool,
            tc.tile_pool(name="attn", bufs=48) as attnpool,
            tc.tile_pool(name="small", bufs=2) as spool,
            tc.tile_pool(name="ysmall", bufs=2) as ypool,
            tc.tile_pool(name="ps_s", bufs=4, space="PSUM") as ps_s,
            tc.tile_pool(name="ps_av", bufs=2, space="PSUM") as ps_av,
            tc.tile_pool(name="ps_g", bufs=2, space="PSUM") as ps_g,
        ):
            packed_sb = wpool.tile([NP, CT, PACK], bf16, tag="packed")
            dma_engines = [nc.sync, nc.scalar, nc.gpsimd]
            HOT = N + 4 * NP  # xt + wqk pairs 0-1: gates qkv(0)
            for ct in range(CT):
                eng = dma_engines[ct % 3]
                eng.dma_start(packed_sb[:, ct, 0:HOT],
                              inp[ct * NP:(ct + 1) * NP, 0:HOT])
            for ct in range(CT):
                eng = dma_engines[ct % 3]
                eng.dma_start(packed_sb[:, ct, HOT:WAVE_A],
                              inp[ct * NP:(ct + 1) * NP, HOT:WAVE_A])
            for ct in range(CT):
                eng = dma_engines[ct % 3]
                eng.dma_start(packed_sb[:, ct, WAVE_A:],
                              inp[ct * NP:(ct + 1) * NP, WAVE_A:])
            bias_sb = wpool.tile([NP, CT], fp32, tag="biassb")
            nc.sync.dma_start(bias_sb[:], bias[:, :])

            xt_sb = packed_sb[:, :, 0:N]
            wqk01 = packed_sb[:, :, N:N + 4 * NP]
            wvt_sb = packed_sb[:, :, N + 4 * NP:WAVE_A]
            wqk27 = packed_sb[:, :, WAVE_A:WAVE_A + 12 * NP]
            wpt_sb = packed_sb[:, :, WAVE_A + 12 * NP:]

            def wqk_ft(ct, ft):
                if ft < 4:
                    return wqk01[:, ct, ft * NP:(ft + 1) * NP]
                return wqk27[:, ct, (ft - 4) * NP:(ft - 3) * NP]

            def body(_it=None):
                q_sb = apool.tile([NP, PAIRS, N], bf16, tag="q")
                k_sb = apool.tile([NP, PAIRS, N], bf16, tag="k")
                vaug_sb = apool.tile([NP, NT, H * (D + 1)], bf16, tag="vaug")
                outT_sb = apool.tile([NP, CT, N], bf16, tag="outT")

                def emit_qkv_chain(ft, nqc):
                    # one 8-chain producing a [128,512] slab of q^T or k^T
                    g = ps_g.tile([NP, NCHUNK], fp32, tag="g", name="qk_g")
                    for ct in range(CT):
                        nc.tensor.matmul(
                            g[:], lhsT=wqk_ft(ct, ft),
                            rhs=xt_sb[:, ct, nqc * NCHUNK:(nqc + 1) * NCHUNK],
                            start=(ct == 0), stop=(ct == CT - 1),
                        )
                    p, which = ft // 2, ft % 2
                    dst = q_sb if which == 0 else k_sb
                    nc.vector.tensor_copy(
                        dst[:, p, nqc * NCHUNK:(nqc + 1) * NCHUNK], g[:])

                def emit_qkv(p):
                    for which in range(2):
                        for nqc in range(NQC):
                            emit_qkv_chain(2 * p + which, nqc)

                def emit_v_chain(nt, fc):
                    # V^T slab [128 keys, 512 chans] -> vaug heads 8fc..8fc+7
                    g = ps_g.tile([NP, NCHUNK], fp32, tag="g", name="v_g")
                    for ct in range(CT):
                        nc.tensor.matmul(
                            g[:], lhsT=xt_sb[:, ct, nt * NP:(nt + 1) * NP],
                            rhs=wvt_sb[:, ct, fc * NCHUNK:(fc + 1) * NCHUNK],
                            start=(ct == 0), stop=(ct == CT - 1),
                        )
                    vrow = vaug_sb[:, nt, :].rearrange("p (h e) -> p h e",
                                                       e=D + 1)
                    hlo = 8 * fc
                    nc.vector.tensor_copy(
                        vrow[:, hlo:hlo + 8, 0:D],
                        g[:].rearrange("p (h e) -> p h e", e=D),
                    )
                    if fc == 0:
                        nc.vector.memset(vrow[:, :, D:D + 1], 1.0)

                def emit_s(p, nkt, atn):
                    # 4 single-group S matmuls + 4 exps per key tile
                    for nqc in range(NQC):
                        for hh in range(2):
                            lo, hi = hh * D, (hh + 1) * D
                            t = ps_s.tile([NP, NCHUNK], fp32, tag="s",
                                          name="s_ps")
                            nc.tensor.matmul(
                                t[:],
                                lhsT=k_sb[lo:hi, p, nkt * NP:(nkt + 1) * NP],
                                rhs=q_sb[lo:hi, p,
                                         nqc * NCHUNK:(nqc + 1) * NCHUNK],
                                start=True, stop=True,
                                tile_position=(hh * D, 0),
                            )
                            a = attnpool.tile([NP, NCHUNK], bf16, tag="attn")
                            nc.scalar.activation(a[:], t[:], Exp,
                                                 scale=float(D) ** -0.5)
                            atn[hh, nqc, nkt] = a

                def emit_av_burst(p, nqc, atn):
                    # per head: one 8-chain over key tiles, then drain
                    for hh in range(2):
                        h = 2 * p + hh
                        av = ps_av.tile([D + 1, NCHUNK], fp32, tag="av",
                                        name="av")
                        for nkt in range(NT):
                            nc.tensor.matmul(
                                av[:],
                                lhsT=vaug_sb[:, nkt,
                                             h * (D + 1):(h + 1) * (D + 1)],
                                rhs=atn[hh, nqc, nkt][:],
                                start=(nkt == 0), stop=(nkt == NT - 1),
                            )
                        recip = spool.tile([1, NCHUNK], fp32, tag="recip")
                        nc.vector.reciprocal(recip[:], av[D:D + 1, :])
                        recip_b = spool.tile([D, NCHUNK], fp32, tag="recipb")
                        nc.gpsimd.partition_broadcast(recip_b[:], recip[:],
                                                      channels=D)
                        dst = outT_sb[hh * D:(hh + 1) * D, p,
                                      nqc * NCHUNK:(nqc + 1) * NCHUNK]
                        nc.vector.tensor_mul(dst, av[0:D, :], recip_b[:])

                # ---- prologue: qkv(0); S(0)+V+qkv(1) interleaved ----
                emit_qkv(0)
                atn_cur = {}
                for nkt in range(NT):
                    emit_s(0, nkt, atn_cur)
                    emit_v_chain(nkt, 0)
                    emit_v_chain(nkt, 1)
                    if nkt % 2 == 1:
                        which, nqc = divmod(nkt // 2, 2)
                        emit_qkv_chain(2 + which, nqc)

                # ---- main loop over pairs ----
                for p in range(PAIRS):
                    atn_nxt = {}
                    for nkt in range(NT):
                        if p + 1 < PAIRS:
                            emit_s(p + 1, nkt, atn_nxt)
                        if p + 2 < PAIRS and nkt % 2 == 1:
                            which, nqc = divmod(nkt // 2, 2)
                            emit_qkv_chain(2 * (p + 2) + which, nqc)
                        if nkt == 2:
                            emit_av_burst(p, 0, atn_cur)
                        elif nkt == 6:
                            emit_av_burst(p, 1, atn_cur)
                    atn_cur = atn_nxt

                # ---- proj + bias + store ----
                # y^T = wpt^T-chunks @ outT: bias is then per-PARTITION and
                # rides the ACT PSUM-evacuation copy (Copy shares Exp's
                # activation table; no table swap)
                Ident = mybir.ActivationFunctionType.Identity
                for ot in range(CT):
                    for nqc in range(NQC):
                        g = ps_g.tile([NP, NCHUNK], fp32, tag="g", name="y_g")
                        for ct in range(CT):
                            nc.tensor.matmul(
                                g[:],
                                lhsT=wpt_sb[:, ct, ot * NP:(ot + 1) * NP],
                                rhs=outT_sb[:, ct,
                                            nqc * NCHUNK:(nqc + 1) * NCHUNK],
                                start=(ct == 0), stop=(ct == CT - 1),
                            )
                        y_sb = ypool.tile([NP, NCHUNK], bf16, tag="ysb",
                                          name="y_sb")
                        nc.scalar.activation(y_sb[:], g[:], Ident,
                                             bias=bias_sb[:, ot:ot + 1])
                        nc.sync.dma_start(
                            out[ot * NP:(ot + 1) * NP,
                                nqc * NCHUNK:(nqc + 1) * NCHUNK], y_sb[:])

            if loop_iters == 1:
                body()
            else:
                with tc.For_i(0, loop_iters, 1) as it:
                    body(it)

    nc.compile()
    _CACHE[loop_iters] = nc
    return nc


def prep_inputs(x, w_qkv, w_proj, b_proj):
    wq, wk, wv = w_qkv[0:DIM], w_qkv[DIM:2 * DIM], w_qkv[2 * DIM:3 * DIM]
    perm = []
    for p in range(PAIRS):
        perm.append(wq[2 * p * D:(2 * p + 2) * D])
        perm.append(wk[2 * p * D:(2 * p + 2) * D])
    wqk_perm = np.concatenate(perm, axis=0)          # [2*DIM, DIM] row-major ft
    wqkT = wqk_perm.T                                 # [DIM, 2*DIM]
    wvT, wpT = wv.T, w_proj.T
    bias_h = np.ascontiguousarray(
        b_proj.reshape(CT, NP).T).astype(np.float32)
    in_maps = []
    for i in range(B):
        xt = x[i].T
        waveA = np.concatenate([xt, wqkT[:, 0:4 * NP], wvT], axis=1)
        waveB = np.concatenate([wqkT[:, 4 * NP:], wpT], axis=1)
        packed = np.ascontiguousarray(
            np.concatenate([waveA, waveB], axis=1).astype(BF16))
        in_maps.append({"inp": packed, "bias": bias_h})
    return in_maps


def kernel(x, w_qkv, w_proj, b_proj):
    from concourse import bass_utils

    x = np.asarray(x, dtype=np.float32)
    w_qkv = np.asarray(w_qkv, dtype=np.float32)
    w_proj = np.asarray(w_proj, dtype=np.float32)
    b_proj = np.asarray(b_proj, dtype=np.float32)
    assert x.shape == (B, N, DIM)

    nc = build(1)
    in_maps = prep_inputs(x, w_qkv, w_proj, b_proj)
    res = bass_utils.run_bass_kernel_spmd(nc, in_maps, core_ids=list(range(B)))
    return np.stack([np.asarray(res.results[i]["out"]).T.astype(np.float32)
                     for i in range(B)], axis=0)
